# revision 1
# baseline (speedup 1.0000x reference)
"""Head-sharded Blenderbot MHA forward, one NeuronCore per 2 heads (v5).

Sharding: D (=16 heads) split across 8 cores -> 128 out-channels (2 heads)
of Q/K/V per core; out_lin is row-parallel (each core computes a full
[B*S, D] partial from its 128 ctx channels); the host sums the 8 fp16
partials (the "all-reduce") and adds out_b. No device-to-device traffic.

Per-core schedule (measured 196.4us vs 275.1us for v3; PE busy ~168us):
 - bf16 datapath (qt, Q/K/V, exp, ctxT, weights); PSUM accumulation fp32;
   fp16 partial output. Matmul rate is unchanged vs f32r (1 cyc/row) but
   DMA halves and DVE gets 2x modes.
 - DMA: HWDGE is one global device (~625ns per dma_start, serial across
   queues) -> few, large descriptors: single-descriptor weight loads
   (host pre-packs the SBUF image), 16 qt transfers for b0 (k-row x
   column-half, arrival-ordered for the first projections), 4 bulk
   transfers for b1, paired fp16 output stores.
 - ACT runs (almost) only the 128 exp instructions (~133us, the second
   wall after PE); every evacuation lives on DVE, the final-pass ctxT
   copy on ACT (idle there).
 - attention is u-serial: per (qc, u) pass scores ping-pong two 2-bank
   PSUM tags, ctx accumulates in one 2-bank tag; the remaining 2-bank
   "P" tag is time-multiplexed by proj chunks, grouped V transposes
   (8 transposes -> one wide copy) and outproj tiles.
 - a global work queue of atomic GROUPS (one P generation each) feeds
   2 items per sk-iteration into the attention passes; groups carry
   gating markers (e.g. outproj needs its qc normalized) and blocked
   groups are SKIPPED by the scanner, so the PE never head-of-line
   stalls. ctx matmuls defer cross-pass until their V-transpose group
   has run (markers guarantee emission order = deadlock freedom).
 - softmax normalization: the ones-row rides the ctx matmul (denominator
   for free); 1/denom via DVE reciprocal straight from PSUM; broadcast
   over 64 rows via DRAM round-trip on idle queues (PE ones-matmul
   broadcast only for the tail-critical last pass); the row-scale mul is
   a deferred pending item so it never blocks the next pass's scores.
 - tc.high_priority nudges the Tile scheduler: scores (+18) and outproj
   matmuls (+15) win ties against filler work.
"""

import functools
from collections import deque
from contextlib import ExitStack

import ml_dtypes
import numpy as np

import concourse.bass as bass
import concourse.tile as tile
from concourse import mybir
from concourse.bass_utils import run_bass_kernel_spmd

B, S, D, H, DH = 2, 2048, 1024, 16, 64
N_CORES = 8
DPC = D // N_CORES        # 128 = 2 heads
BS = B * S
NQC = S // 1024           # 2
NST = S // 128            # 16
NKT = D // 128            # 8

F32 = mybir.dt.float32
F32R = mybir.dt.float32r
F16 = mybir.dt.float16
BF16 = mybir.dt.bfloat16
Act = mybir.ActivationFunctionType
Alu = mybir.AluOpType

FEED_PER_ITER = 2
DRAIN_PER_ITER = 3
PRIO_SCORES = 18
PRIO_OP = 15
PRIO_EVAC = 0
MARKS = []


def _mark(nc, label):
    MARKS.append((int(nc.next_id()), label))


def _split_sync_commands(nc, max_waits=1, max_updates=8):
    for fn in nc.m.functions:
        for bb in fn.blocks:
            new_insts = []
            changed = False
            for inst in bb.instructions:
                si = getattr(inst, "sync_info", None)
                if si is not None:
                    waits = list(si.on_wait or [])
                    if len(waits) > max_waits:
                        for w in waits[:-max_waits]:
                            new_insts.append(mybir.InstNoOp(
                                name=nc.get_next_instruction_name(),
                                ins=[], outs=[], engine=inst.engine,
                                sync_info=mybir.SyncInfo(on_wait=[w], on_update=[]),
                            ))
                        si.on_wait = waits[-max_waits:]
                        changed = True
                    updates = list(si.on_update or [])
                    if len(updates) > max_updates:
                        si.on_update = updates[:max_updates]
                        new_insts.append(inst)
                        new_insts.append(mybir.InstNoOp(
                            name=nc.get_next_instruction_name(),
                            ins=[], outs=[], engine=inst.engine,
                            sync_info=mybir.SyncInfo(
                                on_wait=[], on_update=updates[max_updates:]),
                        ))
                        changed = True
                        continue
                new_insts.append(inst)
            if changed:
                bb.instructions = new_insts


def _bcast_rows(ap, nrows):
    return bass.AP(tensor=ap.tensor, offset=ap.offset,
                   ap=[[0, nrows]] + [list(p) for p in ap.ap[1:]])


def _free_reshape(ap, dims):
    """Reinterpret a [P, N] AP's free dim as nested dims (row-major)."""
    new = [list(ap.ap[0])]
    stride = ap.ap[-1][0]
    total = 1
    for d in dims:
        total *= d
    assert total == ap.ap[-1][1], (dims, ap.ap)
    rem = total
    for d in dims:
        rem //= d
        new.append([stride * rem, d])
    return bass.AP(tensor=ap.tensor, offset=ap.offset, ap=new)


@functools.lru_cache(maxsize=1)
def _build():
    nc = bass.Bass()
    qt_d = nc.dram_tensor("qt", [D, BS], BF16, kind="ExternalInput")
    wq_d = nc.dram_tensor("wq", [128, NKT * DPC], BF16, kind="ExternalInput")
    wk_d = nc.dram_tensor("wk", [128, NKT * DPC], BF16, kind="ExternalInput")
    wv_d = nc.dram_tensor("wv", [128, NKT * DPC], BF16, kind="ExternalInput")
    bq_d = nc.dram_tensor("bq", [DPC, 1], F32, kind="ExternalInput")
    bk_d = nc.dram_tensor("bk", [DPC, 1], F32, kind="ExternalInput")
    bv_d = nc.dram_tensor("bv", [DPC, 1], F32, kind="ExternalInput")
    wo_d = nc.dram_tensor("wo", [DPC, D], BF16, kind="ExternalInput")
    out_d = nc.dram_tensor("out_part", [BS, D], F16, kind="ExternalOutput")
    dn_d = nc.dram_tensor("dn_scratch", [2, S], F32)
    ident_d = nc.inline_tensor(np.eye(128, dtype=np.float32), "ident")
    onesr_d = nc.inline_tensor(np.ones((1, 128), dtype=np.float32), "onesr")

    with tile.TileContext(nc) as tc, ExitStack() as ctx:
        consts = ctx.enter_context(tc.tile_pool(name="consts", bufs=1))
        qt_pool = ctx.enter_context(tc.tile_pool(name="qt", bufs=1))
        projp = ctx.enter_context(tc.tile_pool(name="proj", bufs=2))
        vtp = ctx.enter_context(tc.tile_pool(name="vtp", bufs=2))
        vpool = ctx.enter_context(tc.tile_pool(name="vpool", bufs=2))
        ctxp = ctx.enter_context(tc.tile_pool(name="ctxp", bufs=2))
        expp = ctx.enter_context(tc.tile_pool(name="expp", bufs=20))
        dnp = ctx.enter_context(tc.tile_pool(name="dnp", bufs=1))
        outp = ctx.enter_context(tc.tile_pool(name="outp", bufs=6))
        psp = ctx.enter_context(tc.tile_pool(name="psp", bufs=1, space="PSUM"))

        def ps_tile(shape, tag):
            return psp.tile(shape, F32, tag=tag, name="ps_" + tag)

        # ---- constants ------------------------------------------------------
        # HWDGE is a single global device (~625ns per dma_start instruction,
        # serial across queues): use as FEW dma_start as possible. Weight
        # tensors load in ONE descriptor each via a 3D access pattern.
        def _whole(dram, sb, eng):
            eng.dma_start(out=sb, in_=_free_reshape(dram[:, :], (NKT, DPC)))

        wq_sb = consts.tile([128, NKT, DPC], BF16, tag="wq")
        wk_sb = consts.tile([128, NKT, DPC], BF16, tag="wk")
        wv_sb = consts.tile([128, NKT, DPC], BF16, tag="wv")
        wo_sb = consts.tile([128, D], BF16, tag="wo")
        bq_sb = consts.tile([128, 1], F32, tag="bq")
        bk_sb = consts.tile([128, 1], F32, tag="bk")
        bv_sb = consts.tile([128, 1], F32, tag="bv")
        ident_sb = consts.tile([128, 128], F32R, tag="ident")
        eighth_sb = consts.tile([128, 1], F32, tag="eighth")
        nc.vector.memset(eighth_sb, 0.125)
        zero_sb = consts.tile([128, 1], F32, tag="zero")
        nc.vector.memset(zero_sb, 0.0)
        onesc_sb = consts.tile([1, 128], F32R, tag="onesc")

        def load_consts_head():
            _whole(wk_d, wk_sb, nc.sync)
            _whole(wq_d, wq_sb, nc.scalar)
            nc.scalar.dma_start(out=bk_sb, in_=bk_d[:, :])
            nc.scalar.dma_start(out=bq_sb, in_=bq_d[:, :])
            nc.scalar.dma_start(out=bv_sb, in_=bv_d[:, :])

        def load_consts_rest():
            _whole(wv_d, wv_sb, nc.sync)
            nc.sync.dma_start(out=wo_sb, in_=wo_d[:, :])
            nc.scalar.dma_start(out=ident_sb, in_=ident_d[:, :].bitcast(F32R))
            nc.scalar.dma_start(out=onesc_sb, in_=onesr_d[:, :].bitcast(F32R))

        state = {}

        # ------------------- work queue machinery ---------------------------
        # FWQ is a queue of GROUPS; each group owns one PSUM "P" generation
        # and is atomic (items pop in order, no other group interleaves).
        # A group can carry a `needs` marker: blocked groups are SKIPPED by
        # the scanner so the PE never head-of-line stalls on gated work.
        FWQ = deque()          # groups: [items_deque, needs]
        PROVIDED = set()
        pending = deque()      # (thunk, needs_marker_or_None)
        ACTIVE = [None]
        cur_items = [None]

        def fw(fn, provides=None):
            assert cur_items[0] is not None, "fw() outside a group"
            cur_items[0].append((fn, provides))

        def group(needs=None):
            from contextlib import contextmanager

            @contextmanager
            def _cm():
                items = deque()
                FWQ.append([items, needs])
                prev = cur_items[0]
                cur_items[0] = items
                try:
                    yield
                finally:
                    cur_items[0] = prev
            return _cm()

        def _run_item(g):
            fn, prov = g[0].popleft()
            fn()
            if prov is not None:
                PROVIDED.add(prov)
            if not g[0]:
                if ACTIVE[0] is g:
                    ACTIVE[0] = None
                if g in FWQ:
                    FWQ.remove(g)

        def feed_one():
            g = ACTIVE[0]
            if g is not None:
                if g[1] is None or g[1] in PROVIDED:
                    _run_item(g)
                    return True
                return False
            for i, cand in enumerate(FWQ):
                if i >= 16:
                    break
                if cand[1] is None or cand[1] in PROVIDED:
                    ACTIVE[0] = cand
                    _run_item(cand)
                    return True
            return False

        def feed(n):
            for _ in range(n):
                if not feed_one():
                    return

        def feed_until(marker):
            spins = 0
            while marker not in PROVIDED:
                if not feed_one():
                    try_drain(4)
                    spins += 1
                    assert spins < 2000, f"feed_until({marker}) stuck"

        def try_drain(n):
            done = 0
            while pending and done < n:
                fn, needs = pending[0]
                if needs is not None and needs not in PROVIDED:
                    return
                pending.popleft()
                fn()
                done += 1

        def drain_all():
            while pending:
                fn, needs = pending[0]
                if needs is not None and needs not in PROVIDED:
                    feed_until(needs)
                pending.popleft()
                fn()

        # ------------------------- loads ------------------------------------
        def load_qt_head(b):
            qt_sb = qt_pool.tile([128, NKT, S], BF16, tag="qt")
            state[b, "qt"] = qt_sb
            for i, (eng, k) in enumerate(((nc.sync, 0), (nc.scalar, 1))):
                eng.dma_start(
                    out=qt_sb[:, k, 0:1024],
                    in_=qt_d[k * 128:(k + 1) * 128, b * S: b * S + 1024])

        def load_qt_fine(b, engines, skip_first=0):
            """16 transfers of [128, 1024 cols] (2KB/partition): transfer
            (k, half) fills chunk `half` of k-row. half-0 (= chunk 0) first."""
            qt_sb = state[b, "qt"]
            i = 0
            for h in range(2):
                for k in range(NKT):
                    if h == 0 and k < skip_first:
                        continue
                    engines[i % len(engines)].dma_start(
                        out=qt_sb[:, k, h * 1024:(h + 1) * 1024],
                        in_=qt_d[k * 128:(k + 1) * 128,
                                 b * S + h * 1024: b * S + (h + 1) * 1024])
                    i += 1
            state[b, "qt"] = qt_sb

        def load_qt_bulk(b, eng):
            """4 transfers of [128, 2 k-rows, 1024 cols] via 3D src pattern."""
            qt_sb = qt_pool.tile([128, NKT, S], BF16, tag="qt")
            src0 = qt_d[:, :]
            for h in range(2):
                for kp in range(2):
                    eng.dma_start(
                        out=qt_sb[:, 4 * kp:4 * (kp + 1),
                                  h * 1024:(h + 1) * 1024],
                        in_=bass.AP(
                            tensor=src0.tensor,
                            offset=src0.offset + (4 * kp * 128) * BS
                            + b * S + h * 1024,
                            ap=[[BS, 128], [128 * BS, 4], [1, 1024]]))
            state[b, "qt"] = qt_sb

        # ------------------------- projections ------------------------------
        def alloc_proj(b):
            state[b, "QT"] = projp.tile([128, S], BF16, tag="QT", name="QT")
            state[b, "KT"] = projp.tile([128, S], BF16, tag="KT", name="KT")
            state[b, "VT"] = vtp.tile([128, S], F32R, tag="VT", name="VT")

        def alloc_v(b):
            V = vpool.tile([128, NST, 2, DH + 1], BF16, tag="V", name="V")
            nc.vector.memset(V[:, :, :, DH:DH + 1], 1.0)
            state[b, "V"] = V

        def proj_mm_pair(ps, b, which, pc, k):
            _mark(nc, f"proj_mm[{b}]{which}{pc}")
            qt_sb = state[b, "qt"]
            w_sb = {"q": wq_sb, "k": wk_sb, "v": wv_sb}[which]
            for hh in range(2):
                nc.tensor.matmul(
                    ps[:, hh * 512:(hh + 1) * 512], w_sb[:, k, :],
                    qt_sb[:, k, pc * 1024 + hh * 512: pc * 1024 + (hh + 1) * 512],
                    start=(k == 0), stop=(k == NKT - 1))

        def proj_evac(ps, b, which, pc, on_act=False):
            _mark(nc, f"proj_ev[{b}]{which}{pc}")
            w_b, sc = {"q": (bq_sb, eighth_sb), "k": (bk_sb, None),
                       "v": (bv_sb, None)}[which]
            dst = state[b, {"q": "QT", "k": "KT", "v": "VT"}[which]]
            if on_act:
                assert sc is None
                nc.scalar.activation(dst[:, pc * 1024:(pc + 1) * 1024], ps,
                                     Act.Identity, bias=w_b, scale=1.0)
            elif sc is None:
                nc.vector.tensor_scalar(
                    out=dst[:, pc * 1024:(pc + 1) * 1024], in0=ps,
                    scalar1=w_b, scalar2=None, op0=Alu.add)
            else:
                nc.vector.tensor_scalar(
                    out=dst[:, pc * 1024:(pc + 1) * 1024], in0=ps,
                    scalar1=w_b, scalar2=sc, op0=Alu.add, op1=Alu.mult)

        def fw_proj_chunk(b, which, pc, provides=None):
            holder = {}

            def mm(k):
                if "ps" not in holder:
                    holder["ps"] = ps_tile([128, 1024], "P")
                proj_mm_pair(holder["ps"], b, which, pc, k)

            with group():
                for k in range(NKT):
                    fw(lambda k=k: mm(k))
                fw(lambda: proj_evac(holder["ps"], b, which, pc),
                   provides=provides)

        def proj_chunk_now(b, which, pc):
            ps = ps_tile([128, 1024], "P")
            for k in range(NKT):
                proj_mm_pair(ps, b, which, pc, k)
            proj_evac(ps, b, which, pc)

        # ------------------------- V transpose ------------------------------
        def tr_quad(ps, b, st0):
            _mark(nc, f"tr[{b}]")
            VT = state[b, "VT"]
            for i in range(4):
                nc.tensor.transpose(
                    ps[:, (st0 % 8 + i) * 128:(st0 % 8 + i + 1) * 128
                       ].bitcast(F32R),
                    VT[:, (st0 + i) * 128:(st0 + i + 1) * 128], ident_sb)

        def tr_copy8(ps, b, st0):
            _mark(nc, f"tr[{b}]")
            V = state[b, "V"]
            dst = V[:, st0:st0 + 8, :, 0:DH]
            nc.vector.tensor_copy(dst, _free_reshape(ps[:, :], (8, 2, DH)))

        def fw_tr_group(b, st0):
            holder = {}

            def quad(st):
                if "ps" not in holder:
                    holder["ps"] = ps_tile([128, 1024], "P")
                tr_quad(holder["ps"], b, st)

            def cpy():
                tr_copy8(holder["ps"], b, st0)

            with group():
                fw(lambda: quad(st0))
                fw(lambda: quad(st0 + 4))
                fw(cpy, provides=("trg", b, st0))

        # ------------------------- attention --------------------------------
        def alloc_attn(b):
            state[b, "ctxT"] = ctxp.tile([128, S], BF16, tag="ctxT", name="ctxT")
            # u-rows live at partitions 0 and 32: engine accesses need
            # 32-aligned partition bases
            state[b, "denom"] = dnp.tile([1, 2, S], F32R, tag="denom",
                                          name="denom")
            state[b, "rep"] = dnp.tile([128, S], F32, tag="rep", name="rep")

        def attention_pass(b, qc, u):
            QT, KT, V = state[b, "QT"], state[b, "KT"], state[b, "V"]
            tags = ("sA", "sB")
            pss = {}
            holder = {}
            state["last_holder"] = holder

            def scores(sk):
                _mark(nc, f"scores[{b}]{qc}{u}")
                ps = ps_tile([128, 1024], tags[sk % 2])
                pss[sk] = ps
                with tc.high_priority(offset=PRIO_SCORES):
                    for hh in range(2):
                        nc.tensor.matmul(
                            ps[:, hh * 512:(hh + 1) * 512],
                            KT[u * DH:(u + 1) * DH, sk * 128:(sk + 1) * 128],
                            QT[u * DH:(u + 1) * DH,
                               qc * 1024 + hh * 512:qc * 1024 + (hh + 1) * 512],
                            start=True, stop=True)

            def ctx_mm(sk, e):
                _mark(nc, f"ctx[{b}]{qc}{u}")
                if "c" not in holder:
                    holder["c"] = ps_tile([DH + 1, 1024], "ctx")
                ps_c = holder["c"]
                for hh in range(2):
                    nc.tensor.matmul(
                        ps_c[:, hh * 512:(hh + 1) * 512], V[:, sk, u, :],
                        e[:, hh * 512:(hh + 1) * 512],
                        start=(sk == 0), stop=(sk == NST - 1))

            def ctx_evac():
                _mark(nc, f"ctx_ev[{b}]{qc}{u}")
                ps_c = holder["c"]
                ctxT, denom = state[b, "ctxT"], state[b, "denom"]
                sl = slice(qc * 1024, (qc + 1) * 1024)
                chain_prio = PRIO_EVAC if b == 1 else 0
                with tc.high_priority(offset=chain_prio), \
                        nc.allow_low_precision(
                            reason="f32r is full fp32 bits"):
                    nc.vector.reciprocal(denom[0:1, u, sl],
                                         ps_c[DH:DH + 1, :])
                if (b, qc, u) == (1, 1, 1):
                    # ACT is idle post-attention: copy halves in parallel
                    # with recip so the first tail outproj starts earlier
                    for hh in range(2):
                        cs = slice(qc * 1024 + hh * 512,
                                   qc * 1024 + (hh + 1) * 512)
                        nc.scalar.activation(ctxT[u * DH:(u + 1) * DH, cs],
                                             ps_c[0:DH, hh * 512:(hh + 1) * 512],
                                             Act.Copy, bias=0.0, scale=1.0)
                else:
                    with tc.high_priority(offset=chain_prio):
                        nc.vector.tensor_copy(ctxT[u * DH:(u + 1) * DH, sl],
                                              ps_c[0:DH, :])
                if b == 1 and qc == 1 and u == 1:
                    # tail-critical: broadcast on the PE into the free P tag
                    # (the ctx tag would WAR-wait the ctxT copies)
                    rep = psp.tile([128, 1024], F32, tag="P",
                                   name="ps_rep")
                    for hh in range(2):
                        nc.tensor.matmul(
                            rep[0:DH, hh * 512:(hh + 1) * 512],
                            onesc_sb[0:1, 0:DH],
                            denom[0:1, u,
                                  qc * 1024 + hh * 512:
                                  qc * 1024 + (hh + 1) * 512],
                            start=True, stop=True)
                    holder["rep"] = rep[0:DH, :]
                else:
                    # off the PE: DRAM round-trip broadcast on idle queues
                    repsb = state[b, "rep"]
                    nc.sync.dma_start(out=dn_d[u:u + 1, sl],
                                      in_=denom[0:1, u, sl].bitcast(F32))
                    nc.sync.dma_start(
                        out=repsb[u * DH:(u + 1) * DH, sl],
                        in_=_bcast_rows(dn_d[u:u + 1, sl], DH))
                    holder["rep"] = repsb[u * DH:(u + 1) * DH, sl]

            if qc == 1:
                feed_until(("q1", b))
            scores(0)
            for sk in range(NST):
                ps = pss.pop(sk)
                _mark(nc, f"exp[{b}]{qc}{u}")
                e = expp.tile([128, 1024], BF16, tag="exp", name="exp_t")
                nc.scalar.activation(e, ps, Act.Exp, bias=zero_sb, scale=1.0)
                pending.append(
                    (lambda sk=sk, e=e: ctx_mm(sk, e), ("trg", b, 0 if sk < 8 else 8)))
                if sk + 1 < NST:
                    if sk + 1 == 8 and qc == 0:
                        feed_until(("k1", b))
                    scores(sk + 1)
                feed(FEED_PER_ITER)
                try_drain(DRAIN_PER_ITER)
            pending.append((ctx_evac, None))

        def normalize_u(b, qc, u, holder):
            # only the mul remains deferred; recip+broadcast ran in ctx_evac.
            # Pool does the mul when rep is in SBUF (Pool is otherwise idle
            # and can't read the PSUM rep of the tail pass).
            def run():
                _mark(nc, f"norm[{b}]{qc}{u}")
                ctxT = state[b, "ctxT"]
                sl = slice(qc * 1024, (qc + 1) * 1024)
                if (b, qc, u) == (1, 1, 1):
                    for hh in range(2):
                        cs = slice(qc * 1024 + hh * 512,
                                   qc * 1024 + (hh + 1) * 512)
                        nc.vector.tensor_mul(
                            ctxT[u * DH:(u + 1) * DH, cs],
                            ctxT[u * DH:(u + 1) * DH, cs],
                            holder["rep"][:, hh * 512:(hh + 1) * 512])
                else:
                    nc.gpsimd.tensor_mul(ctxT[u * DH:(u + 1) * DH, sl],
                                         ctxT[u * DH:(u + 1) * DH, sl],
                                         holder["rep"])
                if u == 1:
                    PROVIDED.add(("norm", b, qc))
            pending.append((run, None))

        # ------------------------- out projection ---------------------------
        def outproj_mm(ps, b, st):
            _mark(nc, f"op_mm[{b}]")
            ctxT = state[b, "ctxT"]
            with tc.high_priority(offset=PRIO_OP):
                for oc in range(2):
                    nc.tensor.matmul(ps[:, oc * 512:(oc + 1) * 512],
                                     ctxT[:, st * 128:(st + 1) * 128],
                                     wo_sb[:, oc * 512:(oc + 1) * 512],
                                     start=True, stop=True)

        def outproj_evac(ps, o2, j):
            _mark(nc, "op_ev")
            nc.vector.tensor_copy(o2[:, j, :], ps)

        def outproj_store(o2, b, st0):
            _mark(nc, "op_st")
            # one DMA stores two st tiles: [128, 2, D] -> 256 DRAM rows
            dst = out_d[b * S + st0 * 128: b * S + (st0 + 2) * 128, :]
            nc.sync.dma_start(
                out=bass.AP(tensor=dst.tensor, offset=dst.offset,
                            ap=[[D, 128], [128 * D, 2], [1, D]]),
                in_=o2)

        def fw_outproj(b, sts):
            sts = list(sts)
            assert len(sts) % 2 == 0
            holder = {}

            def mm(st):
                holder["ps"] = ps_tile([128, 1024], "P")
                outproj_mm(holder["ps"], b, st)

            def ev(st, j):
                if j == 0:
                    holder["o2"] = outp.tile([128, 2, D], F16, tag="o",
                                             name="o2")
                outproj_evac(holder["ps"], holder["o2"], j)

            def stre(st0):
                outproj_store(holder["o2"], b, st0)

            for i, st in enumerate(sts):
                with group(needs=("norm", b, st // 8)):
                    fw(lambda st=st: mm(st))
                    fw(lambda st=st, j=i % 2: ev(st, j))
                    if i % 2 == 1:
                        fw(lambda st0=sts[i - 1]: stre(st0))

        # =========================== schedule ===============================
        load_consts_head()
        load_qt_head(0)
        load_qt_fine(0, (nc.sync, nc.scalar), skip_first=2)
        load_consts_rest()
        alloc_proj(0)
        alloc_v(0)
        load_qt_bulk(1, nc.sync)
        # k0/q0 interleaved on the two score tags: both consume the same qt
        # rows as they stream in; evacs run on ACT (idle) and DVE in parallel
        psK = ps_tile([128, 1024], "sA")
        psQ = ps_tile([128, 1024], "sB")
        for k in range(NKT):
            proj_mm_pair(psK, 0, "k", 0, k)
            proj_mm_pair(psQ, 0, "q", 0, k)
        # half-evacs so the first scores can start one half earlier
        KT0, QT0 = state[0, "KT"], state[0, "QT"]
        nc.scalar.activation(KT0[:, 0:512], psK[:, 0:512],
                             Act.Identity, bias=bk_sb, scale=1.0)
        nc.vector.tensor_scalar(out=QT0[:, 0:512], in0=psQ[:, 0:512],
                                scalar1=bq_sb, scalar2=eighth_sb,
                                op0=Alu.add, op1=Alu.mult)
        nc.scalar.activation(KT0[:, 512:1024], psK[:, 512:1024],
                             Act.Identity, bias=bk_sb, scale=1.0)
        nc.vector.tensor_scalar(out=QT0[:, 512:1024], in0=psQ[:, 512:1024],
                                scalar1=bq_sb, scalar2=eighth_sb,
                                op0=Alu.add, op1=Alu.mult)
        alloc_attn(0)

        # b0 leftovers weave into attention(b0) qc0; then b1's first chunks.
        fw_proj_chunk(0, "v", 0)
        fw_tr_group(0, 0)
        fw_proj_chunk(0, "k", 1, provides=("k1", 0))
        fw_proj_chunk(0, "q", 1, provides=("q1", 0))
        fw_proj_chunk(0, "v", 1)
        fw_tr_group(0, 8)

        attention_pass(0, 0, 0)
        normalize_u(0, 0, 0, state["last_holder"])
        attention_pass(0, 0, 1)
        normalize_u(0, 0, 1, state["last_holder"])

        def _alloc_b1():
            alloc_proj(1)
            alloc_v(1)
        with group():
            fw(_alloc_b1)
        fw_proj_chunk(1, "k", 0, provides=("k0", 1))
        fw_proj_chunk(1, "q", 0, provides=("q0", 1))
        fw_outproj(0, range(8))

        attention_pass(0, 1, 0)
        normalize_u(0, 1, 0, state["last_holder"])
        fw_proj_chunk(1, "k", 1, provides=("k1", 1))
        fw_proj_chunk(1, "v", 0)
        fw_tr_group(1, 0)
        attention_pass(0, 1, 1)
        normalize_u(0, 1, 1, state["last_holder"])
        fw_proj_chunk(1, "v", 1)
        fw_tr_group(1, 8)
        fw_proj_chunk(1, "q", 1, provides=("q1", 1))

        # ---- window B: attention(b1) + all outproj + b1 leftovers ----------
        feed_until(("k0", 1))
        feed_until(("q0", 1))
        alloc_attn(1)

        fw_outproj(0, range(8, NST))

        attention_pass(1, 0, 0)
        normalize_u(1, 0, 0, state["last_holder"])
        attention_pass(1, 0, 1)
        normalize_u(1, 0, 1, state["last_holder"])
        fw_outproj(1, range(8))

        attention_pass(1, 1, 0)
        normalize_u(1, 1, 0, state["last_holder"])
        attention_pass(1, 1, 1)
        normalize_u(1, 1, 1, state["last_holder"])
        drain_all()
        spins = 0
        while FWQ:
            if not feed_one():
                try_drain(4)
                spins += 1
                assert spins < 2000, "tail drain stuck"
        # pipelined tail: rotate three free 2-bank tags; evacs alternate
        # DVE/ACT (ACT is idle post-attention); per-st stores
        tail_tags = ("sA", "sB", "P")
        for i, st in enumerate(range(8, NST)):
            ps = ps_tile([128, 1024], tail_tags[i % 3])
            outproj_mm(ps, 1, st)
            o_sb = outp.tile([128, 2, D], F16, tag="o", name="o2")
            if i % 2 == 0:
                nc.vector.tensor_copy(o_sb[:, 0, :], ps)
            else:
                nc.scalar.activation(o_sb[:, 0, :], ps,
                                     Act.Copy, bias=0.0, scale=1.0)
            nc.sync.dma_start(
                out=out_d[S + st * 128: S + (st + 1) * 128, :],
                in_=o_sb[:, 0, :])

    _split_sync_commands(nc)
    return nc


def _sbuf_img(w, sl):
    """[D, DPC] weight slice transposed into its SBUF image [128, NKT*DPC]."""
    bf = ml_dtypes.bfloat16
    wt = w[sl, :].T.reshape(NKT, 128, DPC).transpose(1, 0, 2)
    return np.ascontiguousarray(wt.reshape(128, NKT * DPC)).astype(bf)


def _prepare(query, q_w, q_b, k_w, k_b, v_w, v_b, out_w):
    bf = ml_dtypes.bfloat16
    qt = np.ascontiguousarray(query.reshape(BS, D).T).astype(bf)  # [D, BS]
    in_maps = []
    for c in range(N_CORES):
        sl = slice(c * DPC, (c + 1) * DPC)
        in_maps.append({
            "qt": qt,
            "wq": _sbuf_img(q_w, sl),
            "wk": _sbuf_img(k_w, sl),
            "wv": _sbuf_img(v_w, sl),
            "bq": np.ascontiguousarray(q_b[sl].reshape(DPC, 1)),
            "bk": np.ascontiguousarray(k_b[sl].reshape(DPC, 1)),
            "bv": np.ascontiguousarray(v_b[sl].reshape(DPC, 1)),
            "wo": np.ascontiguousarray(out_w[:, sl].T).astype(bf),
        })
    return in_maps


def kernel(query, mask, q_w, q_b, k_w, k_b, v_w, v_b, out_w, out_b):
    query = np.asarray(query, dtype=np.float32)
    q_w = np.asarray(q_w, dtype=np.float32); q_b = np.asarray(q_b, dtype=np.float32)
    k_w = np.asarray(k_w, dtype=np.float32); k_b = np.asarray(k_b, dtype=np.float32)
    v_w = np.asarray(v_w, dtype=np.float32); v_b = np.asarray(v_b, dtype=np.float32)
    out_w = np.asarray(out_w, dtype=np.float32); out_b = np.asarray(out_b, dtype=np.float32)

    in_maps = _prepare(query, q_w, q_b, k_w, k_b, v_w, v_b, out_w)
    nc = _build()
    res = run_bass_kernel_spmd(nc, in_maps, core_ids=list(range(N_CORES)))
    out = np.zeros((BS, D), dtype=np.float32)
    for c in range(N_CORES):
        out += res.results[c]["out_part"]
    out += out_b[None, :]
    return out.reshape(B, S, D)



# revision 7
# speedup vs baseline: 1.0568x; 1.0568x over previous
"""Head-sharded Blenderbot MHA forward, one NeuronCore per 2 heads (v6).

Sharding: D (=16 heads) split across 8 cores -> 128 out-channels (2 heads)
of Q/K/V per core; out_lin is row-parallel (each core computes a full
[B*S, D] partial from its 128 ctx channels); the host sums the 8 fp16
partials (the "all-reduce") and adds out_b. No device-to-device traffic.

v6 changes vs v5 (195.8us): rebalance engines around the ACT exp wall.
 - ACT runs ONLY the 128 exp instructions during attention (the machine
   floor: 131072 elem/partition @ 1.2GHz = 109us + per-op init). All
   evacuations move to DVE.
 - ctx matmul is FLIPPED: stationary = e-tile column block [keys 128,
   q 128], moving = V [keys 128, DH+1], out = [q 128, DH+1] in PSUM.
   Cost model charges out-free-size (65) instead of moving 1024 per
   sk: 131K -> 67K PE cycles. The denominator rides as V's ones
   column and lands PER-PARTITION (per query), so softmax
   normalization becomes reciprocal [128,8] + one broadcast
   tensor_tensor multiply fused with the evacuation - the v5
   DRAM-round-trip broadcast machinery is gone.
 - ctx comes out [q, dh]-oriented; PE transposes (bf16 identity, 1
   cyc/row) restore ctxT [chan, q] for the out-projection stationary.
 - PSUM: sA/sB score ping-pong (2+2 banks), ctx accumulator (2), P
   (proj chunks / V+ctx transposes / outproj, 2). ctx tile is zeroed
   by two [128,512] matmuls (stationary zeros) so the per-qb
   sub-range accumulation never relies on partial-bank
   start_tensor_calc semantics.
 - PE p-state: only the first matmul after an idle gap pays the mid
   p-state; the work-queue keeps PE saturated with proj/outproj/
   transpose filler so scores stay full-speed.
"""

import functools
from collections import deque
from contextlib import ExitStack

import ml_dtypes
import numpy as np

import concourse.bass as bass
import concourse.tile as tile
from concourse import mybir
from concourse.bass_utils import run_bass_kernel_spmd

B, S, D, H, DH = 2, 2048, 1024, 16, 64
N_CORES = 8
DPC = D // N_CORES        # 128 = 2 heads
BS = B * S
NQC = S // 1024           # 2
NST = S // 128            # 16
NKT = D // 128            # 8
QB = 8                    # 128-query blocks per 1024-query pass

F32 = mybir.dt.float32
F32R = mybir.dt.float32r
F16 = mybir.dt.float16
BF16 = mybir.dt.bfloat16
Act = mybir.ActivationFunctionType
Alu = mybir.AluOpType

FEED_PER_ITER = 2
DRAIN_PER_ITER = 3
PRIO_SCORES = 18
PRIO_OP = 15
PRIO_NORM = 10
MARKS = []


def _mark(nc, label):
    MARKS.append((int(nc.next_id()), label))


def _split_sync_commands(nc, max_waits=1, max_updates=8):
    for fn in nc.m.functions:
        for bb in fn.blocks:
            new_insts = []
            changed = False
            for inst in bb.instructions:
                si = getattr(inst, "sync_info", None)
                if si is not None:
                    waits = list(si.on_wait or [])
                    if len(waits) > max_waits:
                        for w in waits[:-max_waits]:
                            new_insts.append(mybir.InstNoOp(
                                name=nc.get_next_instruction_name(),
                                ins=[], outs=[], engine=inst.engine,
                                sync_info=mybir.SyncInfo(on_wait=[w], on_update=[]),
                            ))
                        si.on_wait = waits[-max_waits:]
                        changed = True
                    updates = list(si.on_update or [])
                    if len(updates) > max_updates:
                        si.on_update = updates[:max_updates]
                        new_insts.append(inst)
                        new_insts.append(mybir.InstNoOp(
                            name=nc.get_next_instruction_name(),
                            ins=[], outs=[], engine=inst.engine,
                            sync_info=mybir.SyncInfo(
                                on_wait=[], on_update=updates[max_updates:]),
                        ))
                        changed = True
                        continue
                new_insts.append(inst)
            if changed:
                bb.instructions = new_insts


def _free_reshape(ap, dims):
    """Reinterpret a [P, N] AP's free dim as nested dims (row-major)."""
    new = [list(ap.ap[0])]
    stride = ap.ap[-1][0]
    total = 1
    for d in dims:
        total *= d
    assert total == ap.ap[-1][1], (dims, ap.ap)
    rem = total
    for d in dims:
        rem //= d
        new.append([stride * rem, d])
    return bass.AP(tensor=ap.tensor, offset=ap.offset, ap=new)


def _bcast_free(ap, n):
    """[P, M] AP -> [P, M, n] with a 0-stride broadcast last dim."""
    return bass.AP(tensor=ap.tensor, offset=ap.offset,
                   ap=[list(p) for p in ap.ap] + [[0, n]])


@functools.lru_cache(maxsize=1)
def _build():
    nc = bass.Bass()
    qt_d = nc.dram_tensor("qt", [D, BS], BF16, kind="ExternalInput")
    wq_d = nc.dram_tensor("wq", [128, NKT * DPC], BF16, kind="ExternalInput")
    wk_d = nc.dram_tensor("wk", [128, NKT * DPC], BF16, kind="ExternalInput")
    wv_d = nc.dram_tensor("wv", [128, NKT * DPC], BF16, kind="ExternalInput")
    bq_d = nc.dram_tensor("bq", [DPC, 1], F32, kind="ExternalInput")
    bk_d = nc.dram_tensor("bk", [DPC, 1], F32, kind="ExternalInput")
    bv_d = nc.dram_tensor("bv", [DPC, 1], F32, kind="ExternalInput")
    wo_d = nc.dram_tensor("wo", [DPC, D], BF16, kind="ExternalInput")
    out_d = nc.dram_tensor("out_part", [BS, D], F16, kind="ExternalOutput")
    ident_d = nc.inline_tensor(
        np.eye(128, dtype=np.float32).astype(ml_dtypes.bfloat16), "ident")

    with tile.TileContext(nc) as tc, ExitStack() as ctx:
        consts = ctx.enter_context(tc.tile_pool(name="consts", bufs=1))
        qt_pool = ctx.enter_context(tc.tile_pool(name="qt", bufs=1))
        projp = ctx.enter_context(tc.tile_pool(name="proj", bufs=2))
        vtp = ctx.enter_context(tc.tile_pool(name="vtp", bufs=2))
        vpool = ctx.enter_context(tc.tile_pool(name="vpool", bufs=2))
        ctxp = ctx.enter_context(tc.tile_pool(name="ctxp", bufs=2))
        expp = ctx.enter_context(tc.tile_pool(name="expp", bufs=18))
        normp = ctx.enter_context(tc.tile_pool(name="normp", bufs=2))
        outp = ctx.enter_context(tc.tile_pool(name="outp", bufs=6))
        psp = ctx.enter_context(tc.tile_pool(name="psp", bufs=1, space="PSUM"))

        def ps_tile(shape, tag):
            return psp.tile(shape, F32, tag=tag, name="ps_" + tag)

        # ---- constants ------------------------------------------------------
        def _whole(dram, sb, eng):
            eng.dma_start(out=sb, in_=_free_reshape(dram[:, :], (NKT, DPC)))

        wq_sb = consts.tile([128, NKT, DPC], BF16, tag="wq")
        wk_sb = consts.tile([128, NKT, DPC], BF16, tag="wk")
        wv_sb = consts.tile([128, NKT, DPC], BF16, tag="wv")
        wo_sb = consts.tile([128, D], BF16, tag="wo")
        bq_sb = consts.tile([128, 1], F32, tag="bq")
        bk_sb = consts.tile([128, 1], F32, tag="bk")
        bv_sb = consts.tile([128, 1], F32, tag="bv")
        ident_sb = consts.tile([128, 128], BF16, tag="ident")
        eighth_sb = consts.tile([128, 1], F32, tag="eighth")
        nc.vector.memset(eighth_sb, 0.125)
        zero_sb = consts.tile([128, 1], F32, tag="zero")
        nc.vector.memset(zero_sb, 0.0)
        zw_sb = consts.tile([128, 512], BF16, tag="zw")
        nc.vector.memset(zw_sb, 0.0)

        def load_consts_head():
            _whole(wk_d, wk_sb, nc.sync)
            _whole(wq_d, wq_sb, nc.scalar)
            nc.scalar.dma_start(out=bk_sb, in_=bk_d[:, :])
            nc.scalar.dma_start(out=bq_sb, in_=bq_d[:, :])
            nc.scalar.dma_start(out=bv_sb, in_=bv_d[:, :])

        def load_consts_rest():
            _whole(wv_d, wv_sb, nc.sync)
            nc.sync.dma_start(out=wo_sb, in_=wo_d[:, :])
            nc.scalar.dma_start(out=ident_sb, in_=ident_d[:, :])

        state = {}

        # ------------------- work queue machinery ---------------------------
        FWQ = deque()          # groups: [items_deque, needs]
        PROVIDED = set()
        pending = deque()      # (thunk, needs_marker_or_None)
        ACTIVE = [None]
        cur_items = [None]

        def fw(fn, provides=None):
            assert cur_items[0] is not None, "fw() outside a group"
            cur_items[0].append((fn, provides))

        def group(needs=None):
            from contextlib import contextmanager

            @contextmanager
            def _cm():
                items = deque()
                FWQ.append([items, needs])
                prev = cur_items[0]
                cur_items[0] = items
                try:
                    yield
                finally:
                    cur_items[0] = prev
            return _cm()

        def _run_item(g):
            fn, prov = g[0].popleft()
            fn()
            if prov is not None:
                if isinstance(prov, list):
                    PROVIDED.update(prov)
                else:
                    PROVIDED.add(prov)
            if not g[0]:
                if ACTIVE[0] is g:
                    ACTIVE[0] = None
                if g in FWQ:
                    FWQ.remove(g)

        def feed_one():
            g = ACTIVE[0]
            if g is not None:
                if g[1] is None or g[1] in PROVIDED:
                    _run_item(g)
                    return True
                return False
            for i, cand in enumerate(FWQ):
                if i >= 16:
                    break
                if cand[1] is None or cand[1] in PROVIDED:
                    ACTIVE[0] = cand
                    _run_item(cand)
                    return True
            return False

        def feed(n):
            for _ in range(n):
                if not feed_one():
                    return

        def feed_until(marker):
            spins = 0
            while marker not in PROVIDED:
                if not feed_one():
                    try_drain(4)
                    spins += 1
                    assert spins < 2000, f"feed_until({marker}) stuck"

        def try_drain(n):
            done = 0
            while pending and done < n:
                fn, needs = pending[0]
                if needs is not None and needs not in PROVIDED:
                    return
                pending.popleft()
                fn()
                done += 1

        def drain_all():
            while pending:
                fn, needs = pending[0]
                if needs is not None and needs not in PROVIDED:
                    feed_until(needs)
                pending.popleft()
                fn()

        # ------------------------- loads ------------------------------------
        def load_qt_head(b):
            qt_sb = qt_pool.tile([128, NKT, S], BF16, tag="qt")
            state[b, "qt"] = qt_sb
            for i, (eng, k) in enumerate(((nc.sync, 0), (nc.scalar, 1))):
                eng.dma_start(
                    out=qt_sb[:, k, 0:1024],
                    in_=qt_d[k * 128:(k + 1) * 128, b * S: b * S + 1024])

        def load_qt_fine(b, engines, skip_first=0):
            qt_sb = state[b, "qt"]
            i = 0
            for h in range(2):
                for k in range(NKT):
                    if h == 0 and k < skip_first:
                        continue
                    engines[i % len(engines)].dma_start(
                        out=qt_sb[:, k, h * 1024:(h + 1) * 1024],
                        in_=qt_d[k * 128:(k + 1) * 128,
                                 b * S + h * 1024: b * S + (h + 1) * 1024])
                    i += 1
            state[b, "qt"] = qt_sb

        def load_qt_bulk(b, eng):
            qt_sb = qt_pool.tile([128, NKT, S], BF16, tag="qt")
            src0 = qt_d[:, :]
            for h in range(2):
                for kp in range(2):
                    eng.dma_start(
                        out=qt_sb[:, 4 * kp:4 * (kp + 1),
                                  h * 1024:(h + 1) * 1024],
                        in_=bass.AP(
                            tensor=src0.tensor,
                            offset=src0.offset + (4 * kp * 128) * BS
                            + b * S + h * 1024,
                            ap=[[BS, 128], [128 * BS, 4], [1, 1024]]))
            state[b, "qt"] = qt_sb

        # ------------------------- projections ------------------------------
        def alloc_proj(b):
            state[b, "QT"] = projp.tile([128, S], BF16, tag="QT", name="QT")
            state[b, "KT"] = projp.tile([128, S], BF16, tag="KT", name="KT")
            state[b, "VT"] = vtp.tile([128, S], BF16, tag="VT", name="VT")

        def alloc_v(b):
            V = vpool.tile([128, NST, 2, DH + 1], BF16, tag="V", name="V")
            nc.vector.memset(V[:, :, :, DH:DH + 1], 1.0)
            state[b, "V"] = V

        def proj_mm_pair(ps, b, which, pc, k):
            _mark(nc, f"proj_mm[{b}]{which}{pc}")
            qt_sb = state[b, "qt"]
            w_sb = {"q": wq_sb, "k": wk_sb, "v": wv_sb}[which]
            for hh in range(2):
                nc.tensor.matmul(
                    ps[:, hh * 512:(hh + 1) * 512], w_sb[:, k, :],
                    qt_sb[:, k, pc * 1024 + hh * 512: pc * 1024 + (hh + 1) * 512],
                    start=(k == 0), stop=(k == NKT - 1))

        def proj_evac(ps, b, which, pc):
            _mark(nc, f"proj_ev[{b}]{which}{pc}")
            w_b, sc = {"q": (bq_sb, eighth_sb), "k": (bk_sb, None),
                       "v": (bv_sb, None)}[which]
            dst = state[b, {"q": "QT", "k": "KT", "v": "VT"}[which]]
            if sc is None:
                nc.vector.tensor_scalar(
                    out=dst[:, pc * 1024:(pc + 1) * 1024], in0=ps,
                    scalar1=w_b, scalar2=None, op0=Alu.add)
            else:
                nc.vector.tensor_scalar(
                    out=dst[:, pc * 1024:(pc + 1) * 1024], in0=ps,
                    scalar1=w_b, scalar2=sc, op0=Alu.add, op1=Alu.mult)

        def fw_proj_chunk(b, which, pc, provides=None):
            holder = {}

            def mm(k):
                if "ps" not in holder:
                    holder["ps"] = ps_tile([128, 1024], "P")
                proj_mm_pair(holder["ps"], b, which, pc, k)

            with group():
                for k in range(NKT):
                    fw(lambda k=k: mm(k))
                fw(lambda: proj_evac(holder["ps"], b, which, pc),
                   provides=provides)

        # ------------------------- V transpose ------------------------------
        def tr_quad(ps, b, st0):
            _mark(nc, f"tr[{b}]")
            VT = state[b, "VT"]
            psb = ps.bitcast(BF16)
            for i in range(4):
                nc.tensor.transpose(
                    psb[:, (st0 % 8 + i) * 128:(st0 % 8 + i + 1) * 128],
                    VT[:, (st0 + i) * 128:(st0 + i + 1) * 128], ident_sb)

        def tr_copy8(ps, b, st0):
            _mark(nc, f"trc[{b}]")
            V = state[b, "V"]
            dst = V[:, st0:st0 + 8, :, 0:DH]
            psb = ps.bitcast(BF16)
            nc.vector.tensor_copy(dst, _free_reshape(psb[:, 0:1024], (8, 2, DH)))

        def fw_tr_group(b, st0):
            holder = {}

            def quad(st):
                if "ps" not in holder:
                    holder["ps"] = ps_tile([128, 1024], "P")
                tr_quad(holder["ps"], b, st)

            def cpy():
                tr_copy8(holder["ps"], b, st0)

            with group():
                fw(lambda: quad(st0))
                fw(lambda: quad(st0 + 4))
                fw(cpy, provides=("trg", b, st0))

        # ------------------------- attention --------------------------------
        def alloc_attn(b):
            state[b, "ctxT"] = ctxp.tile([128, S], BF16, tag="ctxT", name="ctxT")

        PASS_ORDER = [(0, 0, 0), (0, 0, 1), (0, 1, 0), (0, 1, 1),
                      (1, 0, 0), (1, 0, 1), (1, 1, 0), (1, 1, 1)]

        def attention_pass(b, qc, u):
            QT, KT, V = state[b, "QT"], state[b, "KT"], state[b, "V"]
            tags = ("sA", "sB")
            pss = {}
            holder = {}
            pidx = PASS_ORDER.index((b, qc, u))

            def zero_ctx():
                _mark(nc, f"zctx[{b}]{qc}{u}")
                cps = psp.tile([128, QB, 128], F32, tag="ctx", name="ps_ctx")
                holder["c"] = cps
                flat = bass.AP(tensor=cps.tensor, offset=cps.offset,
                               ap=[list(cps.ap[0]), [1, 1024]])
                for hh in range(2):
                    nc.tensor.matmul(
                        flat[:, hh * 512:(hh + 1) * 512], zw_sb[:, 0:128],
                        zw_sb, start=True, stop=False, skip_group_check=True)

            def scores(sk):
                _mark(nc, f"scores[{b}]{qc}{u}")
                ps = ps_tile([128, 1024], tags[sk % 2])
                pss[sk] = ps
                with tc.high_priority(offset=PRIO_SCORES):
                    for hh in range(2):
                        nc.tensor.matmul(
                            ps[:, hh * 512:(hh + 1) * 512],
                            KT[u * DH:(u + 1) * DH, sk * 128:(sk + 1) * 128],
                            QT[u * DH:(u + 1) * DH,
                               qc * 1024 + hh * 512:qc * 1024 + (hh + 1) * 512],
                            start=True, stop=True)

            def ctx_mms(sk, e):
                _mark(nc, f"ctx[{b}]{qc}{u}")
                cps = holder["c"]
                for qb in range(QB):
                    nc.tensor.matmul(
                        cps[:, qb, 0:DH + 1],
                        e[:, qb * 128:(qb + 1) * 128],
                        V[:, sk, u, :],
                        start=False, stop=(sk == NST - 1),
                        skip_group_check=True)

            def norm_chain():
                _mark(nc, f"norm[{b}]{qc}{u}")
                cps = holder["c"]
                rep = normp.tile([128, QB], F32, tag="rep", name="rep")
                ctxn = normp.tile([128, QB, DH], BF16, tag="ctxn", name="ctxn")
                with tc.high_priority(offset=PRIO_NORM):
                    nc.vector.reciprocal(rep, cps[:, :, DH:DH + 1])
                    nc.vector.tensor_tensor(
                        out=ctxn, in0=cps[:, :, 0:DH],
                        in1=_bcast_free(rep[:, :], DH), op=Alu.mult)
                state[b, qc, u, "ctxn"] = ctxn
                PROVIDED.add(("ctxn", b, qc, u))

            if qc == 1:
                feed_until(("q1", b))
            pending.append((zero_ctx, None))
            scores(0)
            for sk in range(NST):
                ps = pss.pop(sk)
                _mark(nc, f"exp[{b}]{qc}{u}")
                # e-tile rotation safety: tile buffers recycle after `bufs`
                # allocations; readers (deferred ctx matmuls) must be EMITTED
                # before the buffer is reused. Force-advance when backlogged.
                spins = 0
                while len(pending) >= 14:
                    h = pending[0][1]
                    if h is not None and h not in PROVIDED:
                        feed_until(h)
                    try_drain(8)
                    spins += 1
                    assert spins < 200, "e backlog drain stuck"
                e = expp.tile([128, 1024], BF16, tag="exp", name="exp_t")
                nc.scalar.activation(e, ps, Act.Exp, bias=zero_sb, scale=1.0)
                pending.append(
                    (lambda sk=sk, e=e: ctx_mms(sk, e),
                     ("trg", b, 0 if sk < 8 else 8)))
                if sk + 1 < NST:
                    if sk + 1 == 8 and qc == 0:
                        feed_until(("k1", b))
                    scores(sk + 1)
                feed(FEED_PER_ITER)
                try_drain(DRAIN_PER_ITER)
            # normp (rep/ctxn) rotation safety: pass N's ctxn buffer is
            # reused at pass N+2 — its readers (the ctx-transpose group of
            # pass N) must be emitted first.
            norm_needs = ("trc",) + PASS_ORDER[pidx - 2] if pidx >= 2 else None
            pending.append((norm_chain, norm_needs))

        # ---------------- ctx transpose (PSUM -> ctxT) ----------------------
        def fw_tr_ctx(b, qc, u):
            holder = {}

            def quads(j):
                _mark(nc, f"ctr[{b}]{qc}{u}")
                if j == 0:
                    holder["ps"] = ps_tile([128, 1024], "P")
                ctxn = state[b, qc, u, "ctxn"]
                psb = holder["ps"].bitcast(BF16)
                for qb in range(4 * j, 4 * j + 4):
                    nc.tensor.transpose(
                        psb[u * DH:(u + 1) * DH, qb * 128:(qb + 1) * 128],
                        ctxn[:, qb, :], ident_sb)

            def ev2():
                _mark(nc, f"cev[{b}]{qc}{u}")
                psb = holder["ps"].bitcast(BF16)
                ctxT = state[b, "ctxT"]
                nc.vector.tensor_copy(
                    ctxT[u * DH:(u + 1) * DH, qc * 1024:(qc + 1) * 1024],
                    psb[u * DH:(u + 1) * DH, 0:1024])

            provs = [("trc", b, qc, u)]
            if u == 1:
                provs.append(("ctxT", b, qc))
            with group(needs=("ctxn", b, qc, u)):
                fw(lambda: quads(0))
                fw(lambda: quads(1))
                fw(ev2, provides=provs)

        # ------------------------- out projection ---------------------------
        def outproj_mm(ps, b, st):
            _mark(nc, f"op_mm[{b}]")
            ctxT = state[b, "ctxT"]
            with tc.high_priority(offset=PRIO_OP):
                for oc in range(2):
                    nc.tensor.matmul(ps[:, oc * 512:(oc + 1) * 512],
                                     ctxT[:, st * 128:(st + 1) * 128],
                                     wo_sb[:, oc * 512:(oc + 1) * 512],
                                     start=True, stop=True)

        def outproj_evac(ps, o2, j):
            _mark(nc, "op_ev")
            nc.vector.tensor_copy(o2[:, j, :], ps)

        def outproj_store(o2, b, st0):
            _mark(nc, "op_st")
            dst = out_d[b * S + st0 * 128: b * S + (st0 + 2) * 128, :]
            nc.sync.dma_start(
                out=bass.AP(tensor=dst.tensor, offset=dst.offset,
                            ap=[[D, 128], [128 * D, 2], [1, D]]),
                in_=o2)

        def fw_outproj(b, sts):
            sts = list(sts)
            assert len(sts) % 2 == 0
            holder = {}

            def mm(st):
                holder["ps"] = ps_tile([128, 1024], "P")
                outproj_mm(holder["ps"], b, st)

            def ev(st, j):
                if j == 0:
                    holder["o2"] = outp.tile([128, 2, D], F16, tag="o",
                                             name="o2")
                outproj_evac(holder["ps"], holder["o2"], j)

            def stre(st0):
                outproj_store(holder["o2"], b, st0)

            for i, st in enumerate(sts):
                with group(needs=("ctxT", b, st // 8)):
                    fw(lambda st=st: mm(st))
                    fw(lambda st=st, j=i % 2: ev(st, j))
                    if i % 2 == 1:
                        fw(lambda st0=sts[i - 1]: stre(st0))

        # =========================== schedule ===============================
        load_consts_head()
        load_qt_head(0)
        load_qt_fine(0, (nc.sync, nc.scalar), skip_first=2)
        load_consts_rest()
        alloc_proj(0)
        alloc_v(0)
        load_qt_bulk(1, nc.sync)
        # k0/q0 interleaved on the two score tags: both consume the same qt
        # rows as they stream in; evacs run on ACT (idle pre-attention) + DVE
        psK = ps_tile([128, 1024], "sA")
        psQ = ps_tile([128, 1024], "sB")
        for k in range(NKT):
            proj_mm_pair(psK, 0, "k", 0, k)
            proj_mm_pair(psQ, 0, "q", 0, k)
        KT0, QT0 = state[0, "KT"], state[0, "QT"]
        nc.scalar.activation(KT0[:, 0:512], psK[:, 0:512],
                             Act.Identity, bias=bk_sb, scale=1.0)
        nc.vector.tensor_scalar(out=QT0[:, 0:512], in0=psQ[:, 0:512],
                                scalar1=bq_sb, scalar2=eighth_sb,
                                op0=Alu.add, op1=Alu.mult)
        nc.scalar.activation(KT0[:, 512:1024], psK[:, 512:1024],
                             Act.Identity, bias=bk_sb, scale=1.0)
        nc.vector.tensor_scalar(out=QT0[:, 512:1024], in0=psQ[:, 512:1024],
                                scalar1=bq_sb, scalar2=eighth_sb,
                                op0=Alu.add, op1=Alu.mult)
        alloc_attn(0)

        # b0 leftovers weave into attention(b0) qc0; then b1's first chunks.
        fw_proj_chunk(0, "v", 0)
        fw_tr_group(0, 0)
        fw_proj_chunk(0, "k", 1, provides=("k1", 0))
        fw_proj_chunk(0, "q", 1, provides=("q1", 0))
        fw_proj_chunk(0, "v", 1)
        fw_tr_group(0, 8)

        attention_pass(0, 0, 0)
        fw_tr_ctx(0, 0, 0)
        attention_pass(0, 0, 1)
        fw_tr_ctx(0, 0, 1)

        def _alloc_b1():
            alloc_proj(1)
            alloc_v(1)
        with group():
            fw(_alloc_b1)
        fw_proj_chunk(1, "k", 0, provides=("k0", 1))
        fw_proj_chunk(1, "q", 0, provides=("q0", 1))
        fw_outproj(0, range(8))

        attention_pass(0, 1, 0)
        fw_tr_ctx(0, 1, 0)
        fw_proj_chunk(1, "k", 1, provides=("k1", 1))
        fw_proj_chunk(1, "v", 0)
        fw_tr_group(1, 0)
        attention_pass(0, 1, 1)
        fw_tr_ctx(0, 1, 1)
        fw_proj_chunk(1, "v", 1)
        fw_tr_group(1, 8)
        fw_proj_chunk(1, "q", 1, provides=("q1", 1))

        # ---- window B: attention(b1) + all outproj + b1 leftovers ----------
        feed_until(("k0", 1))
        feed_until(("q0", 1))
        alloc_attn(1)

        fw_outproj(0, range(8, NST))

        attention_pass(1, 0, 0)
        fw_tr_ctx(1, 0, 0)
        attention_pass(1, 0, 1)
        fw_tr_ctx(1, 0, 1)
        fw_outproj(1, range(8))

        attention_pass(1, 1, 0)
        fw_tr_ctx(1, 1, 0)
        attention_pass(1, 1, 1)
        fw_tr_ctx(1, 1, 1)
        drain_all()
        spins = 0
        while FWQ:
            if not feed_one():
                try_drain(4)
                spins += 1
                assert spins < 2000, "tail drain stuck"
        # pipelined tail: rotate three free 2-bank tags; evacs alternate
        # DVE/ACT (ACT is idle post-attention); per-st stores
        tail_tags = ("sA", "sB", "P")
        for i, st in enumerate(range(8, NST)):
            ps = ps_tile([128, 1024], tail_tags[i % 3])
            outproj_mm(ps, 1, st)
            o_sb = outp.tile([128, 2, D], F16, tag="o", name="o2")
            if i % 2 == 0:
                nc.vector.tensor_copy(o_sb[:, 0, :], ps)
            else:
                nc.scalar.activation(o_sb[:, 0, :], ps,
                                     Act.Copy, bias=0.0, scale=1.0)
            nc.sync.dma_start(
                out=out_d[S + st * 128: S + (st + 1) * 128, :],
                in_=o_sb[:, 0, :])

    _split_sync_commands(nc)
    return nc


def _sbuf_img(w, sl):
    """[D, DPC] weight slice transposed into its SBUF image [128, NKT*DPC]."""
    bf = ml_dtypes.bfloat16
    wt = w[sl, :].T.reshape(NKT, 128, DPC).transpose(1, 0, 2)
    return np.ascontiguousarray(wt.reshape(128, NKT * DPC)).astype(bf)


def _prepare(query, q_w, q_b, k_w, k_b, v_w, v_b, out_w):
    bf = ml_dtypes.bfloat16
    qt = np.ascontiguousarray(query.reshape(BS, D).T).astype(bf)  # [D, BS]
    in_maps = []
    for c in range(N_CORES):
        sl = slice(c * DPC, (c + 1) * DPC)
        in_maps.append({
            "qt": qt,
            "wq": _sbuf_img(q_w, sl),
            "wk": _sbuf_img(k_w, sl),
            "wv": _sbuf_img(v_w, sl),
            "bq": np.ascontiguousarray(q_b[sl].reshape(DPC, 1)),
            "bk": np.ascontiguousarray(k_b[sl].reshape(DPC, 1)),
            "bv": np.ascontiguousarray(v_b[sl].reshape(DPC, 1)),
            "wo": np.ascontiguousarray(out_w[:, sl].T).astype(bf),
        })
    return in_maps


def kernel(query, mask, q_w, q_b, k_w, k_b, v_w, v_b, out_w, out_b):
    query = np.asarray(query, dtype=np.float32)
    q_w = np.asarray(q_w, dtype=np.float32); q_b = np.asarray(q_b, dtype=np.float32)
    k_w = np.asarray(k_w, dtype=np.float32); k_b = np.asarray(k_b, dtype=np.float32)
    v_w = np.asarray(v_w, dtype=np.float32); v_b = np.asarray(v_b, dtype=np.float32)
    out_w = np.asarray(out_w, dtype=np.float32); out_b = np.asarray(out_b, dtype=np.float32)

    in_maps = _prepare(query, q_w, q_b, k_w, k_b, v_w, v_b, out_w)
    nc = _build()
    res = run_bass_kernel_spmd(nc, in_maps, core_ids=list(range(N_CORES)))
    out = np.zeros((BS, D), dtype=np.float32)
    for c in range(N_CORES):
        out += res.results[c]["out_part"]
    out += out_b[None, :]
    return out.reshape(B, S, D)


# revision 14
# speedup vs baseline: 1.0945x; 1.0357x over previous
"""Head-sharded Blenderbot MHA forward, one NeuronCore per 2 heads (v6).

Sharding: D (=16 heads) split across 8 cores -> 128 out-channels (2 heads)
of Q/K/V per core; out_lin is row-parallel (each core computes a full
[B*S, D] partial from its 128 ctx channels); the host sums the 8 fp16
partials (the "all-reduce") and adds out_b. No device-to-device traffic.

v6 changes vs v5 (195.8us): rebalance engines around the ACT exp wall.
 - ACT runs ONLY the 128 exp instructions during attention (the machine
   floor: 131072 elem/partition @ 1.2GHz = 109us + per-op init). All
   evacuations move to DVE.
 - ctx matmul is FLIPPED: stationary = e-tile column block [keys 128,
   q 128], moving = V [keys 128, DH+1], out = [q 128, DH+1] in PSUM.
   Cost model charges out-free-size (65) instead of moving 1024 per
   sk: 131K -> 67K PE cycles. The denominator rides as V's ones
   column and lands PER-PARTITION (per query), so softmax
   normalization becomes reciprocal [128,8] + one broadcast
   tensor_tensor multiply fused with the evacuation - the v5
   DRAM-round-trip broadcast machinery is gone.
 - ctx comes out [q, dh]-oriented; PE transposes (bf16 identity, 1
   cyc/row) restore ctxT [chan, q] for the out-projection stationary.
 - PSUM: sA/sB score ping-pong (2+2 banks), ctx accumulator (2), P
   (proj chunks / V+ctx transposes / outproj, 2). ctx tile is zeroed
   by two [128,512] matmuls (stationary zeros) so the per-qb
   sub-range accumulation never relies on partial-bank
   start_tensor_calc semantics.
 - PE p-state: only the first matmul after an idle gap pays the mid
   p-state; the work-queue keeps PE saturated with proj/outproj/
   transpose filler so scores stay full-speed.
"""

import functools
from collections import deque
from contextlib import ExitStack

import ml_dtypes
import numpy as np

import concourse.bass as bass
import concourse.tile as tile
from concourse import mybir
from concourse.bass_utils import run_bass_kernel_spmd

B, S, D, H, DH = 2, 2048, 1024, 16, 64
N_CORES = 8
DPC = D // N_CORES        # 128 = 2 heads
BS = B * S
NQC = S // 1024           # 2
NST = S // 128            # 16
NKT = D // 128            # 8
QB = 8                    # 128-query blocks per 1024-query pass

F32 = mybir.dt.float32
F32R = mybir.dt.float32r
F16 = mybir.dt.float16
BF16 = mybir.dt.bfloat16
Act = mybir.ActivationFunctionType
Alu = mybir.AluOpType

FEED_PER_ITER = 2
DRAIN_PER_ITER = 3
PRIO_SCORES = 18
PRIO_OP = 15
PRIO_NORM = 10
MARKS = []


def _mark(nc, label):
    MARKS.append((int(nc.next_id()), label))


def _split_sync_commands(nc, max_waits=1, max_updates=8):
    for fn in nc.m.functions:
        for bb in fn.blocks:
            new_insts = []
            changed = False
            for inst in bb.instructions:
                si = getattr(inst, "sync_info", None)
                if si is not None:
                    waits = list(si.on_wait or [])
                    if len(waits) > max_waits:
                        for w in waits[:-max_waits]:
                            new_insts.append(mybir.InstNoOp(
                                name=nc.get_next_instruction_name(),
                                ins=[], outs=[], engine=inst.engine,
                                sync_info=mybir.SyncInfo(on_wait=[w], on_update=[]),
                            ))
                        si.on_wait = waits[-max_waits:]
                        changed = True
                    updates = list(si.on_update or [])
                    if len(updates) > max_updates:
                        si.on_update = updates[:max_updates]
                        new_insts.append(inst)
                        new_insts.append(mybir.InstNoOp(
                            name=nc.get_next_instruction_name(),
                            ins=[], outs=[], engine=inst.engine,
                            sync_info=mybir.SyncInfo(
                                on_wait=[], on_update=updates[max_updates:]),
                        ))
                        changed = True
                        continue
                new_insts.append(inst)
            if changed:
                bb.instructions = new_insts


def _free_reshape(ap, dims):
    """Reinterpret a [P, N] AP's free dim as nested dims (row-major)."""
    new = [list(ap.ap[0])]
    stride = ap.ap[-1][0]
    total = 1
    for d in dims:
        total *= d
    assert total == ap.ap[-1][1], (dims, ap.ap)
    rem = total
    for d in dims:
        rem //= d
        new.append([stride * rem, d])
    return bass.AP(tensor=ap.tensor, offset=ap.offset, ap=new)


def _bcast_free(ap, n):
    """[P, M] AP -> [P, M, n] with a 0-stride broadcast last dim."""
    return bass.AP(tensor=ap.tensor, offset=ap.offset,
                   ap=[list(p) for p in ap.ap] + [[0, n]])


F8 = mybir.dt.float8e4


@functools.lru_cache(maxsize=1)
def _build():
    nc = bass.Bass()
    # qtc[0] = fp8(x^T), qtc[1] = fp8(x^T - qtc[0]): 3-term DoubleRow
    # projection (x8 w8 + x8 wr8 + xr8 w8) carries bf16-level accuracy at
    # 0.75x the bf16 PE cost (2x contraction per pass, 0.5 cyc/row).
    qtc_d = nc.dram_tensor("qtc", [2, D, BS], F8, kind="ExternalInput")
    wq_d = nc.dram_tensor("wq", [2, 128, NKT * DPC], F8, kind="ExternalInput")
    wk_d = nc.dram_tensor("wk", [2, 128, NKT * DPC], F8, kind="ExternalInput")
    wv_d = nc.dram_tensor("wv", [2, 128, NKT * DPC], F8, kind="ExternalInput")
    bq_d = nc.dram_tensor("bq", [DPC, 1], F32, kind="ExternalInput")
    bk_d = nc.dram_tensor("bk", [DPC, 1], F32, kind="ExternalInput")
    bv_d = nc.dram_tensor("bv", [DPC, 1], F32, kind="ExternalInput")
    wo_d = nc.dram_tensor("wo", [DPC, D], BF16, kind="ExternalInput")
    out_d = nc.dram_tensor("out_part", [BS, D], F16, kind="ExternalOutput")
    ident_d = nc.inline_tensor(
        np.eye(128, dtype=np.float32).astype(ml_dtypes.bfloat16), "ident")

    with tile.TileContext(nc) as tc, ExitStack() as ctx:
        consts = ctx.enter_context(tc.tile_pool(name="consts", bufs=1))
        qt_pool = ctx.enter_context(tc.tile_pool(name="qt", bufs=1))
        projp = ctx.enter_context(tc.tile_pool(name="proj", bufs=2))
        vtp = ctx.enter_context(tc.tile_pool(name="vtp", bufs=2))
        vpool = ctx.enter_context(tc.tile_pool(name="vpool", bufs=2))
        ctxp = ctx.enter_context(tc.tile_pool(name="ctxp", bufs=2))
        expp = ctx.enter_context(tc.tile_pool(name="expp", bufs=18))
        normp = ctx.enter_context(tc.tile_pool(name="normp", bufs=2))
        outp = ctx.enter_context(tc.tile_pool(name="outp", bufs=6))
        psp = ctx.enter_context(tc.tile_pool(name="psp", bufs=1, space="PSUM"))

        def ps_tile(shape, tag):
            return psp.tile(shape, F32, tag=tag, name="ps_" + tag)

        # ---- constants ------------------------------------------------------
        def _wpair(dram, sb, eng):
            # [2, 128, NKT*DPC] dram -> [128, 2, NKT, DPC] sbuf in one DMA
            src = dram[:, :, :]
            n = NKT * DPC
            eng.dma_start(out=sb, in_=bass.AP(
                tensor=src.tensor, offset=src.offset,
                ap=[[n, 128], [128 * n, 2], [DPC, NKT], [1, DPC]]))

        wq_sb = consts.tile([128, 2, NKT, DPC], F8, tag="wq")
        wk_sb = consts.tile([128, 2, NKT, DPC], F8, tag="wk")
        wv_sb = consts.tile([128, 2, NKT, DPC], F8, tag="wv")
        wo_sb = consts.tile([128, D], BF16, tag="wo")
        bq_sb = consts.tile([128, 1], F32, tag="bq")
        bk_sb = consts.tile([128, 1], F32, tag="bk")
        bv_sb = consts.tile([128, 1], F32, tag="bv")
        ident_sb = consts.tile([128, 128], BF16, tag="ident")
        zw_sb = consts.tile([128, 512], BF16, tag="zw")
        nc.vector.memset(zw_sb, 0.0)
        sixt_sb = consts.tile([128, 1], F32, tag="sixt")
        nc.vector.memset(sixt_sb, 1.0 / 16.0)
        zero_sb = consts.tile([128, 1], F32, tag="zero")
        nc.vector.memset(zero_sb, 0.0)

        def load_consts_head():
            _wpair(wk_d, wk_sb, nc.sync)
            _wpair(wq_d, wq_sb, nc.scalar)
            nc.scalar.dma_start(out=bk_sb, in_=bk_d[:, :])
            nc.scalar.dma_start(out=bq_sb, in_=bq_d[:, :])
            nc.scalar.dma_start(out=bv_sb, in_=bv_d[:, :])

        def load_consts_rest():
            _wpair(wv_d, wv_sb, nc.scalar)
            nc.scalar.dma_start(out=wo_sb, in_=wo_d[:, :])
            nc.scalar.dma_start(out=ident_sb, in_=ident_d[:, :])

        state = {}

        # ------------------- work queue machinery ---------------------------
        FWQ = deque()          # groups: [items_deque, needs]
        PROVIDED = set()
        pending = deque()      # (thunk, needs_marker_or_None)
        ACTIVE = [None]
        cur_items = [None]

        def fw(fn, provides=None):
            assert cur_items[0] is not None, "fw() outside a group"
            cur_items[0].append((fn, provides))

        def group(needs=None):
            from contextlib import contextmanager

            @contextmanager
            def _cm():
                items = deque()
                FWQ.append([items, needs])
                prev = cur_items[0]
                cur_items[0] = items
                try:
                    yield
                finally:
                    cur_items[0] = prev
            return _cm()

        def _run_item(g):
            fn, prov = g[0].popleft()
            fn()
            if prov is not None:
                if isinstance(prov, list):
                    PROVIDED.update(prov)
                else:
                    PROVIDED.add(prov)
            if not g[0]:
                if ACTIVE[0] is g:
                    ACTIVE[0] = None
                if g in FWQ:
                    FWQ.remove(g)

        def feed_one():
            g = ACTIVE[0]
            if g is not None:
                if g[1] is None or g[1] in PROVIDED:
                    _run_item(g)
                    return True
                return False
            for i, cand in enumerate(FWQ):
                if i >= 16:
                    break
                if cand[1] is None or cand[1] in PROVIDED:
                    ACTIVE[0] = cand
                    _run_item(cand)
                    return True
            return False

        def feed(n):
            for _ in range(n):
                if not feed_one():
                    return

        def feed_until(marker):
            spins = 0
            while marker not in PROVIDED:
                if not feed_one():
                    try_drain(4)
                    spins += 1
                    assert spins < 2000, f"feed_until({marker}) stuck"

        def try_drain(n):
            done = 0
            while pending and done < n:
                fn, needs = pending[0]
                if needs is not None and needs not in PROVIDED:
                    return
                pending.popleft()
                fn()
                done += 1

        def drain_all():
            while pending:
                fn, needs = pending[0]
                if needs is not None and needs not in PROVIDED:
                    feed_until(needs)
                pending.popleft()
                fn()

        # ------------------------- loads ------------------------------------
        # qtc SBUF layout: [128, 2(term), NKT, S] fp8. One DMA per
        # (kt-pair, column-half) moves both terms: pair j lands complete so
        # DoubleRow k-steps can stream behind the transfers.
        def _qt_dma(b, qt_sb, t, k0, nk, h, eng):
            """One 3D transfer: term t, kt rows [k0, k0+nk), column-half h."""
            src = qtc_d[:, :, :]
            eng.dma_start(
                out=qt_sb[:, t, k0:k0 + nk, h * 1024:(h + 1) * 1024],
                in_=bass.AP(
                    tensor=src.tensor,
                    offset=src.offset + t * D * BS + (k0 * 128) * BS
                    + b * S + h * 1024,
                    ap=[[BS, 128], [128 * BS, nk], [1, 1024]]))

        def load_qt_stream(b):
            """b0: column-half 0 pair-by-pair (both terms) so the warmup
            projections stream behind the transfers; then half 1 in bulk."""
            qt_sb = qt_pool.tile([128, 2, NKT, S], F8, tag="qt")
            state[b, "qt"] = qt_sb
            for j in range(NKT // 2):
                for t in range(2):
                    _qt_dma(b, qt_sb, t, 2 * j, 2, 0, nc.sync)
            for t in range(2):
                for kp in range(2):
                    _qt_dma(b, qt_sb, t, 4 * kp, 4, 1, nc.sync)

        def load_qt_bulk(b, eng):
            qt_sb = qt_pool.tile([128, 2, NKT, S], F8, tag="qt")
            state[b, "qt"] = qt_sb
            for h in range(2):
                for t in range(2):
                    for kp in range(2):
                        _qt_dma(b, qt_sb, t, 4 * kp, 4, h, eng)

        # ------------------------- projections ------------------------------
        def alloc_proj(b):
            state[b, "QT"] = projp.tile([128, S], BF16, tag="QT", name="QT")
            state[b, "KT"] = projp.tile([128, S], BF16, tag="KT", name="KT")
            state[b, "VT"] = vtp.tile([128, S], BF16, tag="VT", name="VT")

        def alloc_v(b):
            V = vpool.tile([128, NST, 2, DH + 1], BF16, tag="V", name="V")
            nc.vector.memset(V[:, :, :, DH:DH + 1], 1.0)
            state[b, "V"] = V

        DR = mybir.MatmulPerfMode.DoubleRow

        def proj_mm_j(ps, b, which, pc, j):
            """kt-pair j of a projection chunk: 3 DoubleRow terms x 2 halves."""
            _mark(nc, f"proj_mm[{b}]{which}{pc}")
            qt_sb = state[b, "qt"]
            w_sb = {"q": wq_sb, "k": wk_sb, "v": wv_sb}[which]
            sl = slice(2 * j, 2 * j + 2)
            for hh in range(2):
                cs = slice(pc * 1024 + hh * 512, pc * 1024 + (hh + 1) * 512)
                for t, (wt, xt) in enumerate(
                        ((0, 0), (1, 0), (0, 1))):  # (w8,x8),(wr8,x8),(w8,xr8)
                    nc.tensor.matmul(
                        ps[:, hh * 512:(hh + 1) * 512],
                        w_sb[:, wt, sl, :], qt_sb[:, xt, sl, cs],
                        start=(j == 0 and t == 0), stop=(j == 3 and t == 2),
                        perf_mode=DR, skip_group_check=True)

        def proj_evac(ps, b, which, pc):
            _mark(nc, f"proj_ev[{b}]{which}{pc}")
            w_b = {"q": bq_sb, "k": bk_sb, "v": bv_sb}[which]
            dst = state[b, {"q": "QT", "k": "KT", "v": "VT"}[which]]
            nc.vector.tensor_scalar(
                out=dst[:, pc * 1024:(pc + 1) * 1024], in0=ps,
                scalar1=sixt_sb, scalar2=w_b, op0=Alu.mult, op1=Alu.add)

        def fw_proj_chunk(b, which, pc, provides=None):
            holder = {}

            def mm(j):
                if "ps" not in holder:
                    holder["ps"] = ps_tile([128, 1024], "P")
                proj_mm_j(holder["ps"], b, which, pc, j)

            with group():
                for j in range(NKT // 2):
                    fw(lambda j=j: mm(j))
                fw(lambda: proj_evac(holder["ps"], b, which, pc),
                   provides=provides)

        # ------------------------- V transpose ------------------------------
        def tr_quad(ps, b, st0):
            _mark(nc, f"tr[{b}]")
            VT = state[b, "VT"]
            psb = ps.bitcast(BF16)
            for i in range(4):
                nc.tensor.transpose(
                    psb[:, (st0 % 8 + i) * 128:(st0 % 8 + i + 1) * 128],
                    VT[:, (st0 + i) * 128:(st0 + i + 1) * 128], ident_sb)

        def tr_copy8(ps, b, st0):
            _mark(nc, f"trc[{b}]")
            V = state[b, "V"]
            dst = V[:, st0:st0 + 8, :, 0:DH]
            psb = ps.bitcast(BF16)
            nc.vector.tensor_copy(dst, _free_reshape(psb[:, 0:1024], (8, 2, DH)))

        def fw_tr_group(b, st0):
            holder = {}

            def quad(st):
                if "ps" not in holder:
                    holder["ps"] = ps_tile([128, 1024], "P")
                tr_quad(holder["ps"], b, st)

            def cpy():
                tr_copy8(holder["ps"], b, st0)

            with group():
                fw(lambda: quad(st0))
                fw(lambda: quad(st0 + 4))
                fw(cpy, provides=("trg", b, st0))

        # ------------------------- attention --------------------------------
        def alloc_attn(b):
            state[b, "ctxT"] = ctxp.tile([128, S], BF16, tag="ctxT", name="ctxT")

        PASS_ORDER = [(0, 0, 0), (0, 0, 1), (0, 1, 0), (0, 1, 1),
                      (1, 0, 0), (1, 0, 1), (1, 1, 0), (1, 1, 1)]

        def attention_pass(b, qc, u):
            QT, KT, V = state[b, "QT"], state[b, "KT"], state[b, "V"]
            tags = ("sA", "sB")
            pss = {}
            holder = {}
            pidx = PASS_ORDER.index((b, qc, u))

            def zero_ctx():
                _mark(nc, f"zctx[{b}]{qc}{u}")
                cps = psp.tile([128, QB, 128], F32, tag="ctx", name="ps_ctx")
                holder["c"] = cps
                flat = bass.AP(tensor=cps.tensor, offset=cps.offset,
                               ap=[list(cps.ap[0]), [1, 1024]])
                for hh in range(2):
                    nc.tensor.matmul(
                        flat[:, hh * 512:(hh + 1) * 512], zw_sb[:, 0:128],
                        zw_sb, start=True, stop=False, skip_group_check=True)

            def scores(sk):
                _mark(nc, f"scores[{b}]{qc}{u}")
                ps = ps_tile([128, 1024], tags[sk % 2])
                pss[sk] = ps
                with tc.high_priority(offset=PRIO_SCORES):
                    for hh in range(2):
                        nc.tensor.matmul(
                            ps[:, hh * 512:(hh + 1) * 512],
                            KT[u * DH:(u + 1) * DH, sk * 128:(sk + 1) * 128],
                            QT[u * DH:(u + 1) * DH,
                               qc * 1024 + hh * 512:qc * 1024 + (hh + 1) * 512],
                            start=True, stop=True)

            def ctx_mms(sk, e):
                _mark(nc, f"ctx[{b}]{qc}{u}")
                cps = holder["c"]
                for qb in range(QB):
                    nc.tensor.matmul(
                        cps[:, qb, 0:DH + 1],
                        e[:, qb * 128:(qb + 1) * 128],
                        V[:, sk, u, :],
                        start=False, stop=(sk == NST - 1),
                        skip_group_check=True)

            def norm_chain():
                _mark(nc, f"norm[{b}]{qc}{u}")
                cps = holder["c"]
                rep = normp.tile([128, QB], F32, tag="rep", name="rep")
                ctxn = normp.tile([128, QB, DH], BF16, tag="ctxn", name="ctxn")
                with tc.high_priority(offset=PRIO_NORM):
                    nc.vector.reciprocal(rep, cps[:, :, DH:DH + 1])
                    nc.vector.tensor_tensor(
                        out=ctxn, in0=cps[:, :, 0:DH],
                        in1=_bcast_free(rep[:, :], DH), op=Alu.mult)
                state[b, qc, u, "ctxn"] = ctxn
                PROVIDED.add(("ctxn", b, qc, u))

            if qc == 1:
                feed_until(("q1", b))
            pending.append((zero_ctx, None))
            scores(0)
            for sk in range(NST):
                ps = pss.pop(sk)
                _mark(nc, f"exp[{b}]{qc}{u}")
                # e-tile rotation safety: tile buffers recycle after `bufs`
                # allocations; readers (deferred ctx matmuls) must be EMITTED
                # before the buffer is reused. Force-advance when backlogged.
                spins = 0
                while len(pending) >= 14:
                    h = pending[0][1]
                    if h is not None and h not in PROVIDED:
                        feed_until(h)
                    try_drain(8)
                    spins += 1
                    assert spins < 200, "e backlog drain stuck"
                e = expp.tile([128, 1024], BF16, tag="exp", name="exp_t")
                nc.scalar.activation(e, ps, Act.Exp, bias=zero_sb, scale=1.0)
                pending.append(
                    (lambda sk=sk, e=e: ctx_mms(sk, e),
                     ("trg", b, 0 if sk < 8 else 8)))
                if sk + 1 < NST:
                    if sk + 1 == 8 and qc == 0:
                        feed_until(("k1", b))
                    scores(sk + 1)
                feed(FEED_PER_ITER)
                try_drain(DRAIN_PER_ITER)
            # normp (rep/ctxn) rotation safety: pass N's ctxn buffer is
            # reused at pass N+2 — its readers (the ctx-transpose group of
            # pass N) must be emitted first.
            norm_needs = ("trc",) + PASS_ORDER[pidx - 2] if pidx >= 2 else None
            pending.append((norm_chain, norm_needs))

        # ---------------- ctx transpose (PSUM -> ctxT) ----------------------
        def fw_tr_ctx(b, qc, u):
            holder = {}

            def quads(j):
                _mark(nc, f"ctr[{b}]{qc}{u}")
                if j == 0:
                    holder["ps"] = ps_tile([128, 1024], "P")
                ctxn = state[b, qc, u, "ctxn"]
                psb = holder["ps"].bitcast(BF16)
                for qb in range(4 * j, 4 * j + 4):
                    nc.tensor.transpose(
                        psb[u * DH:(u + 1) * DH, qb * 128:(qb + 1) * 128],
                        ctxn[:, qb, :], ident_sb)

            def ev2():
                _mark(nc, f"cev[{b}]{qc}{u}")
                psb = holder["ps"].bitcast(BF16)
                ctxT = state[b, "ctxT"]
                nc.vector.tensor_copy(
                    ctxT[u * DH:(u + 1) * DH, qc * 1024:(qc + 1) * 1024],
                    psb[u * DH:(u + 1) * DH, 0:1024])

            provs = [("trc", b, qc, u)]
            if u == 1:
                provs.append(("ctxT", b, qc))
            with group(needs=("ctxn", b, qc, u)):
                fw(lambda: quads(0))
                fw(lambda: quads(1))
                fw(ev2, provides=provs)

        # ------------------------- out projection ---------------------------
        def outproj_mm(ps, b, st):
            _mark(nc, f"op_mm[{b}]")
            ctxT = state[b, "ctxT"]
            with tc.high_priority(offset=PRIO_OP):
                for oc in range(2):
                    nc.tensor.matmul(ps[:, oc * 512:(oc + 1) * 512],
                                     ctxT[:, st * 128:(st + 1) * 128],
                                     wo_sb[:, oc * 512:(oc + 1) * 512],
                                     start=True, stop=True)

        def outproj_evac(ps, o2, j):
            _mark(nc, "op_ev")
            nc.vector.tensor_copy(o2[:, j, :], ps)

        def outproj_store(o2, b, st0):
            _mark(nc, "op_st")
            dst = out_d[b * S + st0 * 128: b * S + (st0 + 2) * 128, :]
            nc.sync.dma_start(
                out=bass.AP(tensor=dst.tensor, offset=dst.offset,
                            ap=[[D, 128], [128 * D, 2], [1, D]]),
                in_=o2)

        def fw_outproj(b, sts):
            sts = list(sts)
            assert len(sts) % 2 == 0
            holder = {}

            def mm(st):
                holder["ps"] = ps_tile([128, 1024], "P")
                outproj_mm(holder["ps"], b, st)

            def ev(st, j):
                if j == 0:
                    holder["o2"] = outp.tile([128, 2, D], F16, tag="o",
                                             name="o2")
                outproj_evac(holder["ps"], holder["o2"], j)

            def stre(st0):
                outproj_store(holder["o2"], b, st0)

            for i, st in enumerate(sts):
                with group(needs=("ctxT", b, st // 8)):
                    fw(lambda st=st: mm(st))
                    fw(lambda st=st, j=i % 2: ev(st, j))
                    if i % 2 == 1:
                        fw(lambda st0=sts[i - 1]: stre(st0))

        # =========================== schedule ===============================
        load_consts_head()
        load_qt_stream(0)
        load_consts_rest()
        alloc_proj(0)
        alloc_v(0)
        load_qt_bulk(1, nc.sync)
        # PE p-state ramp burn: the clock needs ~3us of continuous execution
        # to reach 2.4GHz; spend it on dummy matmuls while qt streams in so
        # the real warmup projections run at full speed.
        ramp_ps = psp.tile([128, QB, 128], F32, tag="ctx", name="ps_ramp")
        rflat = bass.AP(tensor=ramp_ps.tensor, offset=ramp_ps.offset,
                        ap=[list(ramp_ps.ap[0]), [1, 1024]])
        for i in range(60):
            nc.tensor.matmul(rflat[:, 0:64], zw_sb[:, 0:128], zw_sb[:, 0:64],
                             start=True, stop=True, skip_group_check=True)
        # k0/q0 interleaved on the two score tags: both consume the same qt
        # pairs as they stream in; evacs run on ACT (idle pre-attention) + DVE
        psK = ps_tile([128, 1024], "sA")
        psQ = ps_tile([128, 1024], "sB")
        for j in range(NKT // 2):
            proj_mm_j(psK, 0, "k", 0, j)
            proj_mm_j(psQ, 0, "q", 0, j)
        KT0, QT0 = state[0, "KT"], state[0, "QT"]
        nc.scalar.activation(KT0[:, 0:512], psK[:, 0:512],
                             Act.Identity, bias=bk_sb, scale=1.0 / 16.0)
        nc.vector.tensor_scalar(out=QT0[:, 0:512], in0=psQ[:, 0:512],
                                scalar1=sixt_sb, scalar2=bq_sb,
                                op0=Alu.mult, op1=Alu.add)
        nc.scalar.activation(KT0[:, 512:1024], psK[:, 512:1024],
                             Act.Identity, bias=bk_sb, scale=1.0 / 16.0)
        nc.vector.tensor_scalar(out=QT0[:, 512:1024], in0=psQ[:, 512:1024],
                                scalar1=sixt_sb, scalar2=bq_sb,
                                op0=Alu.mult, op1=Alu.add)
        alloc_attn(0)

        # b0 leftovers weave into attention(b0) qc0; then b1's first chunks.
        fw_proj_chunk(0, "v", 0)
        fw_tr_group(0, 0)
        fw_proj_chunk(0, "k", 1, provides=("k1", 0))
        fw_proj_chunk(0, "q", 1, provides=("q1", 0))
        fw_proj_chunk(0, "v", 1)
        fw_tr_group(0, 8)

        attention_pass(0, 0, 0)
        fw_tr_ctx(0, 0, 0)
        attention_pass(0, 0, 1)
        fw_tr_ctx(0, 0, 1)

        def _alloc_b1():
            alloc_proj(1)
            alloc_v(1)
        with group():
            fw(_alloc_b1)
        fw_proj_chunk(1, "k", 0, provides=("k0", 1))
        fw_proj_chunk(1, "q", 0, provides=("q0", 1))
        fw_outproj(0, range(8))

        attention_pass(0, 1, 0)
        fw_tr_ctx(0, 1, 0)
        fw_proj_chunk(1, "k", 1, provides=("k1", 1))
        fw_proj_chunk(1, "v", 0)
        fw_tr_group(1, 0)
        attention_pass(0, 1, 1)
        fw_tr_ctx(0, 1, 1)
        fw_proj_chunk(1, "v", 1)
        fw_tr_group(1, 8)
        fw_proj_chunk(1, "q", 1, provides=("q1", 1))

        # ---- window B: attention(b1) + all outproj + b1 leftovers ----------
        feed_until(("k0", 1))
        feed_until(("q0", 1))
        alloc_attn(1)

        fw_outproj(0, range(8, NST))

        attention_pass(1, 0, 0)
        fw_tr_ctx(1, 0, 0)
        attention_pass(1, 0, 1)
        fw_tr_ctx(1, 0, 1)
        fw_outproj(1, range(8))

        attention_pass(1, 1, 0)
        fw_tr_ctx(1, 1, 0)
        attention_pass(1, 1, 1)
        fw_tr_ctx(1, 1, 1)
        drain_all()
        spins = 0
        while FWQ:
            if not feed_one():
                try_drain(4)
                spins += 1
                assert spins < 2000, "tail drain stuck"
        # pipelined tail: rotate three free 2-bank tags; evacs alternate
        # DVE/ACT (ACT is idle post-attention); per-st stores
        tail_tags = ("sA", "sB", "P")
        for i, st in enumerate(range(8, NST)):
            ps = ps_tile([128, 1024], tail_tags[i % 3])
            outproj_mm(ps, 1, st)
            o_sb = outp.tile([128, 2, D], F16, tag="o", name="o2")
            if i % 2 == 0:
                nc.vector.tensor_copy(o_sb[:, 0, :], ps)
            else:
                nc.scalar.activation(o_sb[:, 0, :], ps,
                                     Act.Copy, bias=0.0, scale=1.0)
            nc.sync.dma_start(
                out=out_d[S + st * 128: S + (st + 1) * 128, :],
                in_=o_sb[:, 0, :])

    _split_sync_commands(nc)
    return nc


def _img(w):
    """[DPC, D] float array -> SBUF image [128, NKT*DPC] (same dtype)."""
    wt = w.T.reshape(NKT, 128, DPC).transpose(1, 0, 2)
    return np.ascontiguousarray(wt.reshape(128, NKT * DPC))


def _w_pair_img(w, sl, scale):
    """fp8 weight + residual images, stacked [2, 128, NKT*DPC].

    Values stored at 16x so the residual stays clear of e4m3 subnormals;
    the 1/16 is applied at evacuation time.
    """
    f8 = ml_dtypes.float8_e4m3
    w16 = (w[sl, :].astype(np.float32) * (16.0 * scale))
    w8 = w16.astype(f8)
    wr8 = (w16 - w8.astype(np.float32)).astype(f8)
    return np.ascontiguousarray(np.stack([_img(w8), _img(wr8)]))


def _prepare(query, q_w, q_b, k_w, k_b, v_w, v_b, out_w):
    bf = ml_dtypes.bfloat16
    f8 = ml_dtypes.float8_e4m3
    qt = np.ascontiguousarray(query.reshape(BS, D).T)  # [D, BS] f32
    qt8 = qt.astype(f8)
    qtr8 = (qt - qt8.astype(np.float32)).astype(f8)
    qtc = np.ascontiguousarray(np.stack([qt8, qtr8]))  # [2, D, BS]
    in_maps = []
    for c in range(N_CORES):
        sl = slice(c * DPC, (c + 1) * DPC)
        in_maps.append({
            "qtc": qtc,
            # softmax 1/sqrt(dh) folded into the Q weights/bias
            "wq": _w_pair_img(q_w, sl, 0.125),
            "wk": _w_pair_img(k_w, sl, 1.0),
            "wv": _w_pair_img(v_w, sl, 1.0),
            "bq": np.ascontiguousarray((q_b[sl] * 0.125).reshape(DPC, 1)),
            "bk": np.ascontiguousarray(k_b[sl].reshape(DPC, 1)),
            "bv": np.ascontiguousarray(v_b[sl].reshape(DPC, 1)),
            "wo": np.ascontiguousarray(out_w[:, sl].T).astype(bf),
        })
    return in_maps


def kernel(query, mask, q_w, q_b, k_w, k_b, v_w, v_b, out_w, out_b):
    query = np.asarray(query, dtype=np.float32)
    q_w = np.asarray(q_w, dtype=np.float32); q_b = np.asarray(q_b, dtype=np.float32)
    k_w = np.asarray(k_w, dtype=np.float32); k_b = np.asarray(k_b, dtype=np.float32)
    v_w = np.asarray(v_w, dtype=np.float32); v_b = np.asarray(v_b, dtype=np.float32)
    out_w = np.asarray(out_w, dtype=np.float32); out_b = np.asarray(out_b, dtype=np.float32)

    in_maps = _prepare(query, q_w, q_b, k_w, k_b, v_w, v_b, out_w)
    nc = _build()
    res = run_bass_kernel_spmd(nc, in_maps, core_ids=list(range(N_CORES)))
    out = np.zeros((BS, D), dtype=np.float32)
    for c in range(N_CORES):
        out += res.results[c]["out_part"]
    out += out_b[None, :]
    return out.reshape(B, S, D)
